# revision 13
# baseline (speedup 1.0000x reference)
"""Conv2d 3x3 VALID kernel for Trainium2, batch-sharded across 8 NeuronCores.

Problem: input [32,128,64,64] f32, weights [256,128,3,3] f32 ->
output [32,256,62,62] f32 (stride 1, no padding).

Strategy (per core, 4 images): 1-D Winograd F(2,3) along H + direct taps
along W, matmuls in bf16 (rel-err budget 2e-2, measured ~6e-3).

  For output row pair (2y', 2y'+1), with d_k = in[2y'+k] (rows) and per-kw
  column taps g0..g2 handled by shifted rhs views:
    V0 = d0 - d2, V1 = d1 + d2, V2 = d2 - d1, V3 = d1 - d3   (4 bf16 planes)
    U0 = g0, U1 = (g0+g1+g2)/2, U2 = (g0-g1+g2)/2, U3 = g2   (weights, bf16)
    m_e[y', x] = sum_kw sum_ci U_e,kw[ci,co] * V_e[ci, y', x+kw]  (PE, PSUM)
    out[2y']   = m0 + m1 + m2
    out[2y'+1] = m1 - m2 - m3
  12 matmul passes per 16 output rows instead of the direct method's 18:
  PE row count drops 1.5x.

Pipelining: each chunk (8 y' tiles = 16 out rows) uses ONE 4-bank PSUM tile
(m0..m3), pool bufs=2, so chunk K+1's matmuls run while chunk K's output
transform drains. Engine split per chunk:
  ACT:    one batched copy m0..m2 -> SBUF bf16  ([3,496@512] AP)
  DVE:    t_p=c1+c2, t_m=c1-c2 (bf16 2x), o_even=c0+t_p, o_odd=t_m-m3
  GPSIMD: V planes e0/e1 (from fp32 stage, bf16 out); DVE does e2/e3
All overlap the PE matmul stream (12 MM x 209ns per chunk is the longest).
"""

import numpy as np

import concourse.bass as bass
import concourse.mybir as mybir
import concourse.tile as tile
from concourse import bacc
from concourse.alu_op_type import AluOpType
from concourse.bass_utils import run_bass_kernel_spmd
from concourse.masks import make_identity

F32 = mybir.dt.float32
BF16 = mybir.dt.bfloat16

B, CIN, H, W = 32, 128, 64, 64
COUT, KH, KW = 256, 3, 3
OH, OW = H - KH + 1, W - KW + 1  # 62, 62
N_CORES = 8
BL = B // N_CORES  # 4 images per core

IMG = H * W  # 4096
W_FREE = CIN * KH * KW  # 1152
N_TAPS = KH * KW  # 9
YT = OH // 2  # 31 y' tiles
VPLANE = YT * W  # 1984 elements per V plane
CHUNKS = [(0, 8), (8, 8), (16, 8), (24, 7)]  # (y'0, n_tiles)


def _weight_prep(nc, tc, psum_pool, wtmp_pool, w_d, w_raw, w_t, u_l, ident):
    """DMA weights, PE-transpose taps to [ci, co], build Winograd U planes.

    w_t layout: [ci, tap*256 + h*128 + co]      (fp32, tap = kh*3+kw)
    u_l layout: [ci, (e*3+kw)*256 + h*128 + co] (bf16)
    """
    w_r = w_d.rearrange("co ci kh kw -> co (ci kh kw)")  # [256, 1152]
    nc.sync.dma_start(
        out=w_raw.rearrange("p (h c) -> p h c", h=2),
        in_=w_r.rearrange("(h p) c -> p h c", h=2),
    )
    for h in range(2):
        w_v = w_raw[:, h * W_FREE : (h + 1) * W_FREE].rearrange(
            "p (ci t) -> p t ci", t=N_TAPS
        )
        for t in range(N_TAPS):
            ps = psum_pool.tile(
                [128, 4 * 512], F32, tag="m", name=f"tps_{h}_{t}"
            )
            nc.tensor.transpose(ps[:, :128], w_v[:, t, :], ident)
            nc.vector.tensor_copy(
                w_t[:, t * COUT + h * 128 : t * COUT + h * 128 + 128],
                ps[:, :128],
            )
        # e0/e3 are plain copies of the kh=0 / kh=2 taps (gate the first MMs)
        for kw in range(KW):
            p0 = w_t[:, (0 + kw) * COUT + h * 128 :][:, :128]
            p2 = w_t[:, (6 + kw) * COUT + h * 128 :][:, :128]
            u0 = u_l[:, (0 * 3 + kw) * COUT + h * 128 :][:, :128]
            u3 = u_l[:, (3 * 3 + kw) * COUT + h * 128 :][:, :128]
            nc.vector.tensor_copy(u0, p0)
            nc.vector.tensor_copy(u3, p2)
        for kw in range(KW):
            p0 = w_t[:, (0 + kw) * COUT + h * 128 :][:, :128]
            p1 = w_t[:, (3 + kw) * COUT + h * 128 :][:, :128]
            p2 = w_t[:, (6 + kw) * COUT + h * 128 :][:, :128]
            u1 = u_l[:, (1 * 3 + kw) * COUT + h * 128 :][:, :128]
            u2 = u_l[:, (2 * 3 + kw) * COUT + h * 128 :][:, :128]
            s = wtmp_pool.tile([128, 128], F32, tag="wtmp", name=f"s_{h}_{kw}")
            q = wtmp_pool.tile([128, 128], F32, tag="wtmp", name=f"q_{h}_{kw}")
            nc.vector.tensor_add(s, p0, p2)
            nc.vector.tensor_scalar_mul(q, p1, 0.5)
            nc.vector.scalar_tensor_tensor(
                u1, s, 0.5, q, AluOpType.mult, AluOpType.add
            )
            nc.vector.scalar_tensor_tensor(
                u2, s, 0.5, q, AluOpType.mult, AluOpType.subtract
            )


def _conv_body(nc, tc, out_d, x_d, w_d):
    x_r = x_d.rearrange("b c h w -> b c (h w)")  # [BL, 128, 4096]

    with (
        tc.tile_pool(name="const", bufs=1) as cpool,
        tc.tile_pool(name="psum", bufs=2, space=bass.MemorySpace.PSUM) as psum_pool,
        tc.tile_pool(name="outp", bufs=4) as out_pool,
        tc.tile_pool(name="tmp", bufs=3) as tmp_pool,
    ):
        stage = cpool.tile([128, 2 * IMG], F32)  # rolling fp32 DMA landing
        v_all = cpool.tile([128, BL * 4 * VPLANE], BF16)
        w_raw = cpool.tile([128, 2 * W_FREE], F32)
        w_t = cpool.tile([128, N_TAPS * COUT], F32)
        u_l = cpool.tile([128, 12 * COUT], BF16)
        ident = cpool.tile([128, 128], F32)

        make_identity(nc, ident)
        _weight_prep(nc, tc, psum_pool, tmp_pool, w_d, w_raw, w_t, u_l, ident)

        def prefetch(b):
            """DMA image b (fp32) and build its V planes (bf16)."""
            sl = (b % 2) * IMG
            for c0, c1 in ((0, IMG // 2), (IMG // 2, IMG)):
                nc.sync.dma_start(
                    out=stage[:, sl + c0 : sl + c1], in_=x_r[b][:, c0:c1]
                )
            dv = stage[:, sl : sl + IMG].rearrange("p (r x) -> p r x", x=W)
            # Split in two y' halves so the first half (rows 0..31 == first
            # DMA piece) unblocks the first matmuls early.
            for y0, yn in ((0, 15), (15, 16)):
                r0 = 2 * y0
                rn = 2 * yn

                def rows(k):
                    return dv[:, r0 + k : r0 + k + rn - 1 : 2, :]

                for e, (ra, rb, op) in enumerate(
                    ((0, 2, "sub"), (1, 2, "add"), (2, 1, "sub"), (1, 3, "sub"))
                ):
                    eng = nc.gpsimd if e < 2 else nc.vector
                    vout = v_all[
                        :,
                        (b * 4 + e) * VPLANE + y0 * W : (b * 4 + e) * VPLANE
                        + (y0 + yn) * W,
                    ].rearrange("p (y x) -> p y x", x=W)
                    fn = eng.tensor_add if op == "add" else eng.tensor_sub
                    fn(vout, rows(ra), rows(rb))

        prefetch(0)
        for b in range(BL):
            if b + 1 < BL:
                prefetch(b + 1)
            for h in range(2):
                for y0, ny in CHUNKS:
                    size = ny * OW
                    m = psum_pool.tile([128, 4 * 512], F32, tag="m", name="m")
                    for e in range(4):
                        vv = v_all[
                            :, (b * 4 + e) * VPLANE : (b * 4 + e + 1) * VPLANE
                        ].rearrange("p (y x) -> p y x", x=W)
                        me_v = m[:, e * 512 : e * 512 + size].rearrange(
                            "p (y x) -> p y x", x=OW
                        )
                        for kw in range(KW):
                            lhsT = u_l[:, (e * 3 + kw) * COUT + h * 128 :][:, :128]
                            nc.tensor.matmul(
                                me_v,
                                lhsT,
                                vv[:, y0 : y0 + ny, kw : kw + OW],
                                start=(kw == 0),
                                stop=(kw == KW - 1),
                            )
                    # Output transform: one batched ACT copy + 4 DVE ops.
                    c012 = tmp_pool.tile(
                        [128, 3 * 496], BF16, tag="c012", name="c012"
                    )
                    nc.scalar.copy(
                        c012.rearrange("p (e k) -> p e k", k=496)[:, :, :size],
                        m.rearrange("p (e k) -> p e k", k=512)[:, :3, :size],
                    )
                    t12 = tmp_pool.tile(
                        [128, 2 * 496], BF16, tag="t12", name="t12"
                    )
                    c0 = c012[:, 0:size]
                    c1 = c012[:, 496 : 496 + size]
                    c2 = c012[:, 992 : 992 + size]
                    t_p = t12[:, 0:size]
                    t_m = t12[:, 496 : 496 + size]
                    nc.vector.tensor_add(t_p, c1, c2)
                    nc.vector.tensor_sub(t_m, c1, c2)
                    ot = out_pool.tile([128, 16 * OW], F32, name="ot")
                    ot_v = ot[:, : 2 * ny * OW].rearrange("p (y x) -> p y x", x=OW)

                    def v3(ap):
                        return ap.rearrange("p (y x) -> p y x", x=OW)

                    nc.vector.tensor_add(ot_v[:, 0 : 2 * ny : 2, :], v3(c0), v3(t_p))
                    nc.vector.tensor_sub(
                        ot_v[:, 1 : 2 * ny : 2, :],
                        v3(t_m),
                        v3(m[:, 3 * 512 : 3 * 512 + size]),
                    )
                    nc.sync.dma_start(
                        out=out_d[
                            b,
                            h * 128 : (h + 1) * 128,
                            2 * y0 : 2 * y0 + 2 * ny,
                            :,
                        ],
                        in_=ot_v,
                    )


def build_module():
    nc = bacc.Bacc(
        "TRN2", target_bir_lowering=False, debug=False, num_devices=N_CORES
    )
    x_d = nc.dram_tensor(
        "input_image", [BL, CIN, H, W], F32, kind="ExternalInput"
    ).ap()
    w_d = nc.dram_tensor("weights", [COUT, CIN, KH, KW], F32, kind="ExternalInput").ap()
    out_d = nc.dram_tensor("out", [BL, COUT, OH, OW], F32, kind="ExternalOutput").ap()
    with tile.TileContext(nc) as tc:
        _conv_body(nc, tc, out_d, x_d, w_d)
    nc.compile()
    return nc


_NC_CACHE = {}


def _get_module():
    if "m" not in _NC_CACHE:
        _NC_CACHE["m"] = build_module()
    return _NC_CACHE["m"]


def kernel(input_image: np.ndarray, weights: np.ndarray) -> np.ndarray:
    input_image = np.ascontiguousarray(input_image, dtype=np.float32)
    weights = np.ascontiguousarray(weights, dtype=np.float32)
    nc = _get_module()
    in_maps = [
        {
            "input_image": input_image[i * BL : (i + 1) * BL],
            "weights": weights,
        }
        for i in range(N_CORES)
    ]
    res = run_bass_kernel_spmd(nc, in_maps, list(range(N_CORES))).results
    return np.concatenate([r["out"] for r in res], axis=0)


# revision 15
# speedup vs baseline: 1.1226x; 1.1226x over previous
"""Conv2d 3x3 VALID kernel for Trainium2, batch-sharded across 8 NeuronCores.

Problem: input [32,128,64,64] f32, weights [256,128,3,3] f32 ->
output [32,256,62,62] f32 (stride 1, no padding).

Strategy (per core, 4 images): 1-D Winograd F(2,3) along H + direct taps
along W, matmuls in bf16 (rel-err budget 2e-2, measured ~6e-3).

  For output row pair (2y', 2y'+1), with d_k = in[2y'+k] (rows) and per-kw
  column taps g0..g2 handled by shifted rhs views:
    V0 = d0 - d2, V1 = d1 + d2, V2 = d2 - d1, V3 = d1 - d3   (4 bf16 planes)
    U0 = g0, U1 = (g0+g1+g2)/2, U2 = (g0-g1+g2)/2, U3 = g2   (weights, bf16)
    m_e[y', x] = sum_kw sum_ci U_e,kw[ci,co] * V_e[ci, y', x+kw]  (PE, PSUM)
    out[2y']   = m0 + m1 + m2
    out[2y'+1] = m1 - m2 - m3
  12 matmul passes per 16 output rows instead of the direct method's 18:
  PE row count drops 1.5x.

Pipelining: each chunk (8 y' tiles = 16 out rows) uses ONE 4-bank PSUM tile
(m0..m3), pool bufs=2, so chunk K+1's matmuls run while chunk K's output
transform drains. Engine split per chunk:
  ACT:    one batched copy m0..m2 -> SBUF bf16  ([3,496@512] AP)
  DVE:    t_p=c1+c2, t_m=c1-c2 (bf16 2x), o_even=c0+t_p, o_odd=t_m-m3
  GPSIMD: V planes e0/e1 (from fp32 stage, bf16 out); DVE does e2/e3
All overlap the PE matmul stream (12 MM x 209ns per chunk is the longest).
"""

import numpy as np

import concourse.bass as bass
import concourse.mybir as mybir
import concourse.tile as tile
from concourse import bacc
from concourse.alu_op_type import AluOpType
from concourse.bass_utils import run_bass_kernel_spmd
from concourse.masks import make_identity

F32 = mybir.dt.float32
BF16 = mybir.dt.bfloat16

B, CIN, H, W = 32, 128, 64, 64
COUT, KH, KW = 256, 3, 3
OH, OW = H - KH + 1, W - KW + 1  # 62, 62
N_CORES = 8
BL = B // N_CORES  # 4 images per core

IMG = H * W  # 4096
W_FREE = CIN * KH * KW  # 1152
N_TAPS = KH * KW  # 9
YT = OH // 2  # 31 y' tiles
VPLANE = YT * W  # 1984 elements per V plane
CHUNKS = [(0, 8), (8, 8), (16, 8), (24, 7)]  # (y'0, n_tiles)


def _weight_prep(nc, tc, psum_pool, wtmp_pool, w_d, w_raw, w_t, u_l, ident):
    """DMA weights, PE-transpose taps to [ci, co], build Winograd U planes.

    w_t layout: [ci, tap*256 + h*128 + co]      (fp32, tap = kh*3+kw)
    u_l layout: [ci, (e*3+kw)*256 + h*128 + co] (bf16)
    """
    w_r = w_d.rearrange("co ci kh kw -> co (ci kh kw)")  # [256, 1152]
    nc.sync.dma_start(
        out=w_raw.rearrange("p (h c) -> p h c", h=2),
        in_=w_r.rearrange("(h p) c -> p h c", h=2),
    )
    for h in range(2):
        w_v = w_raw[:, h * W_FREE : (h + 1) * W_FREE].rearrange(
            "p (ci t) -> p t ci", t=N_TAPS
        )
        for t in range(N_TAPS):
            ps = psum_pool.tile(
                [128, 4 * 512], F32, tag="m", name=f"tps_{h}_{t}"
            )
            nc.tensor.transpose(ps[:, :128], w_v[:, t, :], ident)
            nc.vector.tensor_copy(
                w_t[:, t * COUT + h * 128 : t * COUT + h * 128 + 128],
                ps[:, :128],
            )
        # e0/e3 are plain copies of the kh=0 / kh=2 taps (gate the first MMs)
        for kw in range(KW):
            p0 = w_t[:, (0 + kw) * COUT + h * 128 :][:, :128]
            p2 = w_t[:, (6 + kw) * COUT + h * 128 :][:, :128]
            u0 = u_l[:, (0 * 3 + kw) * COUT + h * 128 :][:, :128]
            u3 = u_l[:, (3 * 3 + kw) * COUT + h * 128 :][:, :128]
            nc.vector.tensor_copy(u0, p0)
            nc.vector.tensor_copy(u3, p2)
        for kw in range(KW):
            p0 = w_t[:, (0 + kw) * COUT + h * 128 :][:, :128]
            p1 = w_t[:, (3 + kw) * COUT + h * 128 :][:, :128]
            p2 = w_t[:, (6 + kw) * COUT + h * 128 :][:, :128]
            u1 = u_l[:, (1 * 3 + kw) * COUT + h * 128 :][:, :128]
            u2 = u_l[:, (2 * 3 + kw) * COUT + h * 128 :][:, :128]
            s = wtmp_pool.tile([128, 128], F32, tag="wtmp", name=f"s_{h}_{kw}")
            q = wtmp_pool.tile([128, 128], F32, tag="wtmp", name=f"q_{h}_{kw}")
            nc.vector.tensor_add(s, p0, p2)
            nc.vector.tensor_scalar_mul(q, p1, 0.5)
            nc.vector.scalar_tensor_tensor(
                u1, s, 0.5, q, AluOpType.mult, AluOpType.add
            )
            nc.vector.scalar_tensor_tensor(
                u2, s, 0.5, q, AluOpType.mult, AluOpType.subtract
            )


def _conv_body(nc, tc, out_d, x_d, w_d):
    x_r = x_d.rearrange("b c h w -> b c (h w)")  # [BL, 128, 4096]

    with (
        tc.tile_pool(name="const", bufs=1) as cpool,
        tc.tile_pool(name="psum", bufs=2, space=bass.MemorySpace.PSUM) as psum_pool,
        tc.tile_pool(name="outp", bufs=4) as out_pool,
        tc.tile_pool(name="tmp", bufs=3) as tmp_pool,
    ):
        stage = cpool.tile([128, 2 * IMG], F32)  # rolling fp32 DMA landing
        v_all = cpool.tile([128, BL * 4 * VPLANE], BF16)
        w_raw = cpool.tile([128, 2 * W_FREE], F32)
        w_t = cpool.tile([128, N_TAPS * COUT], F32)
        u_l = cpool.tile([128, 12 * COUT], BF16)
        ident = cpool.tile([128, 128], F32)

        make_identity(nc, ident)
        _weight_prep(nc, tc, psum_pool, tmp_pool, w_d, w_raw, w_t, u_l, ident)

        def prefetch(b):
            """DMA image b (fp32) and build its V planes (bf16)."""
            sl = (b % 2) * IMG
            for c0, c1 in ((0, IMG // 2), (IMG // 2, IMG)):
                nc.sync.dma_start(
                    out=stage[:, sl + c0 : sl + c1], in_=x_r[b][:, c0:c1]
                )
            dv = stage[:, sl : sl + IMG].rearrange("p (r x) -> p r x", x=W)
            # Split in two y' halves so the first half (rows 0..31 == first
            # DMA piece) unblocks the first matmuls early.
            for y0, yn in ((0, 15), (15, 16)):
                r0 = 2 * y0
                rn = 2 * yn

                def rows(k):
                    return dv[:, r0 + k : r0 + k + rn - 1 : 2, :]

                for e, (ra, rb, op) in enumerate(
                    ((0, 2, "sub"), (1, 2, "add"), (2, 1, "sub"), (1, 3, "sub"))
                ):
                    eng = nc.gpsimd if e < 3 else nc.vector
                    vout = v_all[
                        :,
                        (b * 4 + e) * VPLANE + y0 * W : (b * 4 + e) * VPLANE
                        + (y0 + yn) * W,
                    ].rearrange("p (y x) -> p y x", x=W)
                    fn = eng.tensor_add if op == "add" else eng.tensor_sub
                    fn(vout, rows(ra), rows(rb))

        prefetch(0)
        for b in range(BL):
            if b + 1 < BL:
                prefetch(b + 1)
            for h in range(2):
                for y0, ny in CHUNKS:
                    size = ny * OW
                    m = psum_pool.tile([128, 4 * 512], F32, tag="m", name="m")
                    for e in range(4):
                        vv = v_all[
                            :, (b * 4 + e) * VPLANE : (b * 4 + e + 1) * VPLANE
                        ].rearrange("p (y x) -> p y x", x=W)
                        me_v = m[:, e * 512 : e * 512 + size].rearrange(
                            "p (y x) -> p y x", x=OW
                        )
                        for kw in range(KW):
                            lhsT = u_l[:, (e * 3 + kw) * COUT + h * 128 :][:, :128]
                            nc.tensor.matmul(
                                me_v,
                                lhsT,
                                vv[:, y0 : y0 + ny, kw : kw + OW],
                                start=(kw == 0),
                                stop=(kw == KW - 1),
                            )
                    # Output transform: one batched ACT copy of all four m
                    # planes (sole PSUM reader, so the 4-bank slot recycles
                    # after ~1.8us < the 2.5us of the next chunk's matmuls),
                    # then 4 SBUF-only DVE ops.
                    cm = tmp_pool.tile(
                        [128, 4 * 496], BF16, tag="cm", name="cm"
                    )
                    nc.scalar.copy(
                        cm.rearrange("p (e k) -> p e k", k=496)[:, :, :size],
                        m.rearrange("p (e k) -> p e k", k=512)[:, :, :size],
                    )
                    t12 = tmp_pool.tile(
                        [128, 2 * 496], BF16, tag="t12", name="t12"
                    )
                    c0 = cm[:, 0:size]
                    c1 = cm[:, 496 : 496 + size]
                    c2 = cm[:, 992 : 992 + size]
                    c3 = cm[:, 1488 : 1488 + size]
                    t_p = t12[:, 0:size]
                    t_m = t12[:, 496 : 496 + size]
                    nc.vector.tensor_add(t_p, c1, c2)
                    nc.vector.tensor_sub(t_m, c1, c2)
                    ot = out_pool.tile([128, 16 * OW], F32, name="ot")
                    ot_v = ot[:, : 2 * ny * OW].rearrange("p (y x) -> p y x", x=OW)

                    def v3(ap):
                        return ap.rearrange("p (y x) -> p y x", x=OW)

                    nc.vector.tensor_add(ot_v[:, 0 : 2 * ny : 2, :], v3(c0), v3(t_p))
                    nc.vector.tensor_sub(
                        ot_v[:, 1 : 2 * ny : 2, :], v3(t_m), v3(c3)
                    )
                    nc.sync.dma_start(
                        out=out_d[
                            b,
                            h * 128 : (h + 1) * 128,
                            2 * y0 : 2 * y0 + 2 * ny,
                            :,
                        ],
                        in_=ot_v,
                    )


def build_module():
    nc = bacc.Bacc(
        "TRN2", target_bir_lowering=False, debug=False, num_devices=N_CORES
    )
    x_d = nc.dram_tensor(
        "input_image", [BL, CIN, H, W], F32, kind="ExternalInput"
    ).ap()
    w_d = nc.dram_tensor("weights", [COUT, CIN, KH, KW], F32, kind="ExternalInput").ap()
    out_d = nc.dram_tensor("out", [BL, COUT, OH, OW], F32, kind="ExternalOutput").ap()
    with tile.TileContext(nc) as tc:
        _conv_body(nc, tc, out_d, x_d, w_d)
    nc.compile()
    return nc


_NC_CACHE = {}


def _get_module():
    if "m" not in _NC_CACHE:
        _NC_CACHE["m"] = build_module()
    return _NC_CACHE["m"]


def kernel(input_image: np.ndarray, weights: np.ndarray) -> np.ndarray:
    input_image = np.ascontiguousarray(input_image, dtype=np.float32)
    weights = np.ascontiguousarray(weights, dtype=np.float32)
    nc = _get_module()
    in_maps = [
        {
            "input_image": input_image[i * BL : (i + 1) * BL],
            "weights": weights,
        }
        for i in range(N_CORES)
    ]
    res = run_bass_kernel_spmd(nc, in_maps, list(range(N_CORES))).results
    return np.concatenate([r["out"] for r in res], axis=0)


# revision 17
# speedup vs baseline: 1.4785x; 1.3171x over previous
"""Conv2d 3x3 VALID kernel for Trainium2, batch-sharded across 8 NeuronCores.

Problem: input [32,128,64,64] f32, weights [256,128,3,3] f32 ->
output [32,256,62,62] f32 (stride 1, no padding).

Strategy (per core, 4 images): 1-D Winograd F(2,3) along H + direct taps
along W, matmuls in bf16 (rel-err budget 2e-2, measured ~6e-3).

  For output row pair (2y', 2y'+1), with d_k = in[2y'+k] (rows) and per-kw
  column taps g0..g2 handled by shifted rhs views:
    V0 = d0 - d2, V1 = d1 + d2, V2 = d2 - d1, V3 = d1 - d3   (4 bf16 planes)
    U0 = g0, U1 = (g0+g1+g2)/2, U2 = (g0-g1+g2)/2, U3 = g2   (weights, bf16)
    m_e[y', x] = sum_kw sum_ci U_e,kw[ci,co] * V_e[ci, y', x+kw]  (PE, PSUM)
    out[2y']   = m0 + m1 + m2
    out[2y'+1] = m1 - m2 - m3
  12 matmul passes per 16 output rows instead of the direct method's 18:
  PE row count drops 1.5x.

Engine budget per image (~20us of matmuls):
  ACT:    fp32->bf16 input converts (2) + one 4-plane PSUM->SBUF bf16 copy
          per chunk (the only PSUM reader, so the 4-bank PSUM slot recycles
          in ~1.8us < 2.5us of the next chunk's matmuls; pool bufs=2).
  DVE:    V planes (bf16 2x_1P) + output combines batched over chunk PAIRS
          (t_p=c1+c2, t_m=c1-c2, o_even=c0+t_p, o_odd=t_m-c3; FD=992).
  GPSIMD: nothing. Its SBUF port is shared with the Vector engine and any
          streaming GPSIMD op slows concurrent DVE ops ~8x (measured).
"""

import numpy as np

import concourse.bass as bass
import concourse.mybir as mybir
import concourse.tile as tile
from concourse import bacc
from concourse.alu_op_type import AluOpType
from concourse.bass_utils import run_bass_kernel_spmd
from concourse.masks import make_identity

F32 = mybir.dt.float32
BF16 = mybir.dt.bfloat16

B, CIN, H, W = 32, 128, 64, 64
COUT, KH, KW = 256, 3, 3
OH, OW = H - KH + 1, W - KW + 1  # 62, 62
N_CORES = 8
BL = B // N_CORES  # 4 images per core

IMG = H * W  # 4096
W_FREE = CIN * KH * KW  # 1152
N_TAPS = KH * KW  # 9
YT = OH // 2  # 31 y' tiles
VPLANE = YT * W  # 1984 elements per V plane
CHUNKS = [(0, 8), (8, 8), (16, 8), (24, 7)]  # (y'0, n_tiles)
CPAIRS = [CHUNKS[:2], CHUNKS[2:]]


def _weight_prep(nc, tc, psum_pool, wtmp_pool, w_d, w_raw, w_t, u_l, ident):
    """DMA weights, PE-transpose taps to [ci, co], build Winograd U planes.

    w_t layout: [ci, tap*256 + h*128 + co]      (fp32, tap = kh*3+kw)
    u_l layout: [ci, (e*3+kw)*256 + h*128 + co] (bf16)
    """
    w_r = w_d.rearrange("co ci kh kw -> co (ci kh kw)")  # [256, 1152]
    nc.sync.dma_start(
        out=w_raw.rearrange("p (h c) -> p h c", h=2),
        in_=w_r.rearrange("(h p) c -> p h c", h=2),
    )
    for h in range(2):
        w_v = w_raw[:, h * W_FREE : (h + 1) * W_FREE].rearrange(
            "p (ci t) -> p t ci", t=N_TAPS
        )
        for t in range(N_TAPS):
            ps = psum_pool.tile(
                [128, 4 * 512], F32, tag="m", name=f"tps_{h}_{t}"
            )
            nc.tensor.transpose(ps[:, :128], w_v[:, t, :], ident)
            nc.vector.tensor_copy(
                w_t[:, t * COUT + h * 128 : t * COUT + h * 128 + 128],
                ps[:, :128],
            )
        # e0/e3 are plain copies of the kh=0 / kh=2 taps (gate the first MMs)
        for kw in range(KW):
            p0 = w_t[:, (0 + kw) * COUT + h * 128 :][:, :128]
            p2 = w_t[:, (6 + kw) * COUT + h * 128 :][:, :128]
            u0 = u_l[:, (0 * 3 + kw) * COUT + h * 128 :][:, :128]
            u3 = u_l[:, (3 * 3 + kw) * COUT + h * 128 :][:, :128]
            nc.vector.tensor_copy(u0, p0)
            nc.vector.tensor_copy(u3, p2)
        for kw in range(KW):
            p0 = w_t[:, (0 + kw) * COUT + h * 128 :][:, :128]
            p1 = w_t[:, (3 + kw) * COUT + h * 128 :][:, :128]
            p2 = w_t[:, (6 + kw) * COUT + h * 128 :][:, :128]
            u1 = u_l[:, (1 * 3 + kw) * COUT + h * 128 :][:, :128]
            u2 = u_l[:, (2 * 3 + kw) * COUT + h * 128 :][:, :128]
            s = wtmp_pool.tile([128, 128], F32, tag="wtmp", name=f"s_{h}_{kw}")
            q = wtmp_pool.tile([128, 128], F32, tag="wtmp", name=f"q_{h}_{kw}")
            nc.vector.tensor_add(s, p0, p2)
            nc.vector.tensor_scalar_mul(q, p1, 0.5)
            nc.vector.scalar_tensor_tensor(
                u1, s, 0.5, q, AluOpType.mult, AluOpType.add
            )
            nc.vector.scalar_tensor_tensor(
                u2, s, 0.5, q, AluOpType.mult, AluOpType.subtract
            )


def _conv_body(nc, tc, out_d, x_d, w_d):
    x_r = x_d.rearrange("b c h w -> b c (h w)")  # [BL, 128, 4096]

    with (
        tc.tile_pool(name="const", bufs=1) as cpool,
        tc.tile_pool(name="psum", bufs=2, space=bass.MemorySpace.PSUM) as psum_pool,
        tc.tile_pool(name="outp", bufs=3) as out_pool,
        tc.tile_pool(name="tmp", bufs=3) as tmp_pool,
    ):
        stage = cpool.tile([128, 2 * IMG], F32)  # rolling fp32 DMA landing
        in_bf = cpool.tile([128, 2 * IMG], BF16)  # rolling bf16 image
        v_all = cpool.tile([128, 2 * 4 * VPLANE], BF16)  # rolling V planes
        w_raw = cpool.tile([128, 2 * W_FREE], F32)
        w_t = cpool.tile([128, N_TAPS * COUT], F32)
        u_l = cpool.tile([128, 12 * COUT], BF16)
        ident = cpool.tile([128, 128], F32)

        make_identity(nc, ident)
        _weight_prep(nc, tc, psum_pool, tmp_pool, w_d, w_raw, w_t, u_l, ident)

        def prefetch(b):
            """DMA image b, ACT-convert to bf16, DVE-build V planes (bf16)."""
            sl = (b % 2) * IMG
            for c0, c1 in ((0, IMG // 2), (IMG // 2, IMG)):
                nc.sync.dma_start(
                    out=stage[:, sl + c0 : sl + c1], in_=x_r[b][:, c0:c1]
                )
                nc.scalar.copy(
                    in_bf[:, sl + c0 : sl + c1], stage[:, sl + c0 : sl + c1]
                )
            dv = in_bf[:, sl : sl + IMG].rearrange("p (r x) -> p r x", x=W)
            # For image 0, split each plane in two y' halves so the first
            # half (rows 0..31 == the first converted piece) unblocks the
            # first matmuls early; later images build whole planes.
            spans = ((0, 15), (15, 16)) if b == 0 else ((0, 31),)
            for y0, yn in spans:
                r0 = 2 * y0
                rn = 2 * yn

                def rows(k):
                    return dv[:, r0 + k : r0 + k + rn - 1 : 2, :]

                for e, (ra, rb, op) in enumerate(
                    ((0, 2, "sub"), (1, 2, "add"), (2, 1, "sub"), (1, 3, "sub"))
                ):
                    vout = v_all[
                        :,
                        ((b % 2) * 4 + e) * VPLANE
                        + y0 * W : ((b % 2) * 4 + e) * VPLANE
                        + (y0 + yn) * W,
                    ].rearrange("p (y x) -> p y x", x=W)
                    fn = nc.vector.tensor_add if op == "add" else nc.vector.tensor_sub
                    fn(vout, rows(ra), rows(rb))

        prefetch(0)
        for b in range(BL):
            if b + 1 < BL:
                prefetch(b + 1)
            for h in range(2):
                for cpair in CPAIRS:
                    cms = []
                    for pi, (y0, ny) in enumerate(cpair):
                        size = ny * OW
                        m = psum_pool.tile(
                            [128, 4 * 512], F32, tag="m", name=f"m_{pi}"
                        )
                        for e in range(4):
                            vv = v_all[
                                :,
                                ((b % 2) * 4 + e)
                                * VPLANE : ((b % 2) * 4 + e + 1)
                                * VPLANE,
                            ].rearrange("p (y x) -> p y x", x=W)
                            me_v = m[:, e * 512 : e * 512 + size].rearrange(
                                "p (y x) -> p y x", x=OW
                            )
                            for kw in range(KW):
                                lhsT = u_l[:, (e * 3 + kw) * COUT + h * 128 :][
                                    :, :128
                                ]
                                nc.tensor.matmul(
                                    me_v,
                                    lhsT,
                                    vv[:, y0 : y0 + ny, kw : kw + OW],
                                    start=(kw == 0),
                                    stop=(kw == KW - 1),
                                )
                        # Sole PSUM reader: batched 4-plane bf16 copy.
                        cm = tmp_pool.tile(
                            [128, 4 * 496], BF16, tag=f"cm{pi}", name=f"cm_{pi}"
                        )
                        nc.scalar.copy(
                            cm.rearrange("p (e k) -> p e k", k=496)[:, :, :size],
                            m.rearrange("p (e k) -> p e k", k=512)[:, :, :size],
                        )
                        cms.append(cm)

                    # Output transform per chunk (SBUF-only DVE ops).
                    t12 = tmp_pool.tile(
                        [128, 2 * 2 * 496], BF16, tag="t12", name="t12"
                    )
                    ot = out_pool.tile([128, 2 * 16 * OW], F32, name="ot")
                    for pi, (y0, ny) in enumerate(cpair):
                        size = ny * OW
                        cm = cms[pi]
                        c0 = cm[:, 0:size]
                        c1 = cm[:, 496 : 496 + size]
                        c2 = cm[:, 992 : 992 + size]
                        c3 = cm[:, 1488 : 1488 + size]
                        t_p = t12[:, pi * 992 : pi * 992 + size]
                        t_m = t12[:, pi * 992 + 496 : pi * 992 + 496 + size]
                        nc.vector.tensor_add(t_p, c1, c2)
                        nc.vector.tensor_sub(t_m, c1, c2)
                        ot_v = ot[
                            :, pi * 16 * OW : pi * 16 * OW + 2 * ny * OW
                        ].rearrange("p (y x) -> p y x", x=OW)

                        def v3(ap):
                            return ap.rearrange("p (y x) -> p y x", x=OW)

                        nc.vector.tensor_add(
                            ot_v[:, 0 : 2 * ny : 2, :], v3(c0), v3(t_p)
                        )
                        nc.vector.tensor_sub(
                            ot_v[:, 1 : 2 * ny : 2, :], v3(t_m), v3(c3)
                        )
                        nc.sync.dma_start(
                            out=out_d[
                                b,
                                h * 128 : (h + 1) * 128,
                                2 * y0 : 2 * y0 + 2 * ny,
                                :,
                            ],
                            in_=ot_v,
                        )


def build_module():
    nc = bacc.Bacc(
        "TRN2", target_bir_lowering=False, debug=False, num_devices=N_CORES
    )
    x_d = nc.dram_tensor(
        "input_image", [BL, CIN, H, W], F32, kind="ExternalInput"
    ).ap()
    w_d = nc.dram_tensor("weights", [COUT, CIN, KH, KW], F32, kind="ExternalInput").ap()
    out_d = nc.dram_tensor("out", [BL, COUT, OH, OW], F32, kind="ExternalOutput").ap()
    with tile.TileContext(nc) as tc:
        _conv_body(nc, tc, out_d, x_d, w_d)
    nc.compile()
    return nc


_NC_CACHE = {}


def _get_module():
    if "m" not in _NC_CACHE:
        _NC_CACHE["m"] = build_module()
    return _NC_CACHE["m"]


def kernel(input_image: np.ndarray, weights: np.ndarray) -> np.ndarray:
    input_image = np.ascontiguousarray(input_image, dtype=np.float32)
    weights = np.ascontiguousarray(weights, dtype=np.float32)
    nc = _get_module()
    in_maps = [
        {
            "input_image": input_image[i * BL : (i + 1) * BL],
            "weights": weights,
        }
        for i in range(N_CORES)
    ]
    res = run_bass_kernel_spmd(nc, in_maps, list(range(N_CORES))).results
    return np.concatenate([r["out"] for r in res], axis=0)
